# revision 1
# baseline (speedup 1.0000x reference)
"""MultiHeadAttention (N=2, S=T=4096, E=768, H=12, DH=64) on 8 NeuronCores.

Sharding: batch x head-group. Core k handles batch n=k//4 and heads
[3g, 3g+3) with g=k%4. Each core computes Q/K/V projections for its 3
heads, causal/masked attention, and a partial output projection
(tensor-parallel split of Wo along the head dim, bias/4 folded in).
Host sums the 4 partials per batch (Megatron-style row-parallel unshard).

Device layout: scores are computed transposed (S^T[t, q]) so the
attention @ V matmul consumes exp-scores directly as the moving operand,
and an extra ones-column in V yields the softmax denominators for free.
The attention mask is classified host-side into per-(512q x 128t) blocks:
all-zero blocks are skipped, all-one blocks need no masking, and mixed
blocks are shipped (transposed, bf16) and multiplied in after exp.

Attention is processed in query-block PAIRS (qb0, qb1) with the t-loop
innermost, so each kt / v stationary tile serves two matmuls, halving
PE LdWeights traffic. Softmax normalization uses reciprocal_approx_fast
plus a DMA partition-broadcast (instead of slow DVE reciprocal + PE
broadcast matmul), and the PSUM accumulator is copied to SBUF as soon
as a head finishes so the bank frees for the next head.
"""

import os
import sys

for _p in ("/opt/trn_rl_repo",):
    if _p not in sys.path and os.path.isdir(_p):
        sys.path.append(_p)

import numpy as np
import ml_dtypes

import concourse.bass as bass
import concourse.tile as tile
from concourse import mybir
from concourse.bass_utils import run_bass_kernel_spmd

BF16 = ml_dtypes.bfloat16
NP8 = ml_dtypes.float8_e4m3
F32 = mybir.dt.float32
BF = mybir.dt.bfloat16
FP8 = mybir.dt.float8e4
DR = mybir.MatmulPerfMode.DoubleRow

N, S, T, E, H = 2, 4096, 4096, 768, 12
DH = 64
HPC = 3            # heads per core
NH3 = HPC * DH     # 192
QB = 512           # query block (free dim of score tiles)
TB = 128           # key/t block (partition dim of score tiles)
NQB = S // QB      # 8
NTB = T // TB      # 32
EC = E // 128      # 6 contraction chunks for projections
EC2 = E // 256     # 3 DoubleRow chunks (2 x 128 rows each)
SCALE = 32.0       # fp8 weight prescale; folded out via Wo/SCALE + exp scale


# ---------------------------------------------------------------------------
# This walrus build rejects instructions carrying more than MAX_WAITS sem
# waits. After Tile scheduling, hoist excess waits onto single-wait nops
# inserted just before the offending instruction on the same engine
# (same-engine program order makes this semantics-preserving).
MAX_WAITS = 1


def _split_waits(nc, limit=MAX_WAITS):
    ctr = 0
    for bb in nc.m.functions[0].blocks:
        out = []
        dirty = False
        for inst in bb.instructions:
            si = inst.sync_info
            nw = len(si.on_wait) if (si and si.on_wait) else 0
            if nw > limit:
                waits = list(si.on_wait)
                for w in waits[:-limit]:
                    ctr += 1
                    out.append(
                        mybir.InstNoOp(
                            name=f"bass_waitsplit_{ctr}",
                            engine=inst.engine,
                            sync_info=mybir.SyncInfo(on_wait=[w], on_update=[]),
                            bass_nofuse=True,
                        )
                    )
                inst.sync_info = mybir.SyncInfo(
                    on_wait=waits[-limit:], on_update=list(si.on_update or [])
                )
                dirty = True
            out.append(inst)
        if dirty:
            bb.instructions = out
    return nc


# ---------------------------------------------------------------------------
def _mask_plan(mask: np.ndarray):
    """Classify mask into (NQB x NTB) blocks.

    Returns (plan, packed) where plan[qb] is a list of (tb, mix_idx|None)
    for blocks that contribute, and packed is [n_mixed, TB, QB] bf16 of
    transposed mixed blocks.
    """
    m = mask.reshape(NQB, QB, NTB, TB)
    sums = m.astype(np.int64).sum(axis=(1, 3))  # [NQB, NTB]
    full = QB * TB
    plan = []
    packed = []
    for qb in range(NQB):
        row = []
        for tb in range(NTB):
            s = int(sums[qb, tb])
            if s == 0:
                continue
            if s == full:
                row.append((tb, None, 0, QB))
            else:
                blk = m[qb, :, tb, :]  # [QB, TB]
                alive = blk.any(axis=1)
                qlo = int(np.argmax(alive))
                qhi = QB - int(np.argmax(alive[::-1]))
                # round to 128-col granularity to keep APs simple
                qlo = (qlo // 128) * 128
                qhi = -(-qhi // 128) * 128
                if (blk[qlo:qhi, :] != 0).all():
                    row.append((tb, None, qlo, qhi))
                else:
                    row.append((tb, len(packed), qlo, qhi))
                    packed.append(
                        np.ascontiguousarray(blk.T).astype(BF16)
                    )
        plan.append(row)
    if packed:
        packed_arr = np.stack(packed)
    else:
        packed_arr = np.zeros((1, TB, QB), BF16)
    return plan, packed_arr


# ---------------------------------------------------------------------------
def _build(plan, n_mixed):
    nc = bass.Bass("TRN2", target_bir_lowering=False, debug=False, num_devices=8)

    qt = nc.declare_dram_parameter("qt", [E, S], BF, isOutput=False)
    kt = nc.declare_dram_parameter("kt", [E, S], BF, isOutput=False)
    vt = nc.declare_dram_parameter("vt", [E, S], BF, isOutput=False)
    wq = nc.declare_dram_parameter("wq", [E, NH3], BF, isOutput=False)
    wk = nc.declare_dram_parameter("wk", [E, NH3], BF, isOutput=False)
    wv = nc.declare_dram_parameter("wv", [E, NH3], BF, isOutput=False)
    bqkv = nc.declare_dram_parameter("bqkv", [3, NH3], F32, isOutput=False)
    wo = nc.declare_dram_parameter("wo", [NH3 + 1, E], BF, isOutput=False)
    maskt = nc.declare_dram_parameter(
        "maskt", [max(n_mixed, 1), TB, QB], BF, isOutput=False
    )
    out = nc.declare_dram_parameter("out", [S, E], F32, isOutput=True)

    with tile.TileContext(nc) as tc:
        with (
            tc.tile_pool(name="consts", bufs=1) as consts,
            tc.tile_pool(name="persist", bufs=1) as persist,
            tc.tile_pool(name="xblk", bufs=2) as xpool,
            tc.tile_pool(name="ptmp", bufs=2) as tpool,
            tc.tile_pool(name="vblk", bufs=3) as vpool,
            tc.tile_pool(name="ea", bufs=4) as epool,
            tc.tile_pool(name="ys", bufs=2) as ypool,
            tc.tile_pool(name="recs", bufs=2) as rpool,
            tc.tile_pool(name="osb", bufs=2) as opool,
        ):
            # ---- constants --------------------------------------------------
            wq_sb = consts.tile([128, EC, NH3], BF, tag="wq")
            nc.sync.dma_start(out=wq_sb, in_=wq.rearrange("(c p) m -> p c m", p=128))
            wk_sb = consts.tile([128, EC, NH3], BF, tag="wk")
            nc.sync.dma_start(out=wk_sb, in_=wk.rearrange("(c p) m -> p c m", p=128))
            wv_sb = consts.tile([128, EC, NH3], BF, tag="wv")
            nc.sync.dma_start(out=wv_sb, in_=wv.rearrange("(c p) m -> p c m", p=128))
            wo0 = consts.tile([128, E], BF, tag="wo0")
            nc.gpsimd.dma_start(out=wo0, in_=wo[0:128, :])
            wo1 = consts.tile([NH3 + 1 - 128, E], BF, tag="wo1")
            nc.gpsimd.dma_start(out=wo1, in_=wo[128 : NH3 + 1, :])
            # per-partition (d) bias scalars for Q/K: [64, tensor, head]
            bias_sb = consts.tile([DH, 3, HPC], F32, tag="bias")
            nc.gpsimd.dma_start(
                out=bias_sb, in_=bqkv.rearrange("t (h d) -> d t h", d=DH)
            )
            # V bias broadcast along partitions: [128, HPC, DH]
            bv_sb = consts.tile([128, HPC, DH], F32, tag="bv")
            bsrc = bqkv[2:3, :]
            nc.gpsimd.dma_start(
                out=bv_sb,
                in_=bass.AP(
                    tensor=bsrc.tensor,
                    offset=bsrc.offset,
                    ap=[[0, 128], [DH, HPC], [1, DH]],
                ),
            )
            ones_sb = consts.tile([1, DH], F32, tag="ones")
            nc.vector.memset(ones_sb, 1.0)
            bias2 = consts.tile([128, 3], F32, tag="bias2")
            nc.vector.tensor_copy(bias2[0:DH, :], bias_sb[:, :, 0])
            nc.vector.tensor_copy(bias2[DH:128, :], bias_sb[:, :, 1])
            if n_mixed:
                # off the SP HWDGE queue so it doesn't delay the first
                # projection block loads
                mk_sb = consts.tile([TB, n_mixed, QB], BF, tag="mk")
                nc.gpsimd.dma_start(out=mk_sb, in_=maskt.rearrange("m p q -> p m q"))

            # ---- Q/K projections -> QT_sb/KT_sb [64, HPC, S] bf16 ----------
            # Q^T/K^T duplicated across both partition halves so score
            # matmuls can alternate PE row-tiles (0,0)/(64,0) and overlap
            qt_sb = persist.tile([128, HPC, S], BF, tag="qt_sb")
            kt_sb = persist.tile([128, HPC, S], BF, tag="kt_sb")
            v_all = persist.tile([128, NTB, HPC, DH + 1], BF, tag="v_all")
            with tc.tile_pool(name="ps_qk", bufs=2, space="PSUM") as ps_qk:
                for ti, (xsrc, wsb, dst) in enumerate(
                    ((qt, wq_sb, qt_sb), (kt, wk_sb, kt_sb))
                ):
                    for sb in range(NQB):
                        xblk = xpool.tile([128, EC, QB], BF, tag="xblk")
                        (nc.sync if sb % 2 == 0 else nc.scalar).dma_start(
                            out=xblk,
                            in_=xsrc[:, sb * QB : (sb + 1) * QB].rearrange(
                                "(c p) s -> p c s", p=128
                            ),
                        )
                        ps01 = ps_qk.tile([128, QB], F32, tag="ps01")
                        ps2p = ps_qk.tile([DH, QB], F32, tag="ps2")
                        for c in range(EC):
                            nc.tensor.matmul(
                                ps01,
                                wsb[:, c, 0:128],
                                xblk[:, c, :],
                                start=(c == 0),
                                stop=(c == EC - 1),
                            )
                        for c in range(EC):
                            nc.tensor.matmul(
                                ps2p,
                                wsb[:, c, 128:NH3],
                                xblk[:, c, :],
                                start=(c == 0),
                                stop=(c == EC - 1),
                            )
                        for h in range(HPC):
                            psrc = ps01[h * DH : (h + 1) * DH, :] if h < 2 else ps2p
                            nc.vector.tensor_scalar_add(
                                dst[0:DH, h, sb * QB : (sb + 1) * QB],
                                psrc,
                                bias_sb[:, ti : ti + 1, h],
                            )
                    # replicate into the upper partition half for row-tiling
                    nc.vector.tensor_copy(
                        dst[DH:128, :, :], dst[0:DH, :, :]
                    )

                # ---- V projection -> v_all [128, NTB, HPC, DH+1] bf16 ------
                nc.vector.memset(v_all[:, :, :, DH : DH + 1], 1.0)
                for tb in range(NTB):
                    vblk = vpool.tile([128, EC, TB], BF, tag="vblk")
                    (nc.sync if tb % 2 == 0 else nc.scalar).dma_start(
                        out=vblk,
                        in_=vt[:, tb * TB : (tb + 1) * TB].rearrange(
                            "(c p) t -> p c t", p=128
                        ),
                    )
                    psv = ps_qk.tile([128, HPC, DH], F32, tag="psv")
                    for c in range(EC):
                        nc.tensor.matmul(
                            psv,
                            vblk[:, c, :],
                            wv_sb[:, c, :],
                            start=(c == 0),
                            stop=(c == EC - 1),
                        )
                    nc.vector.tensor_add(v_all[:, tb, :, 0:DH], psv, bv_sb)

            # ---- attention + output projection ------------------------------
            # PSUM budget (8 banks): sps [128,1024] x2 bufs = 4, ytp0/ytp1
            # [65,512] x1 buf = 2, po [128,768] x1 buf = 2.
            with (
                tc.tile_pool(name="ps_s", bufs=2, space="PSUM") as ps_s,
                tc.tile_pool(name="ps_y", bufs=1, space="PSUM") as ps_y,
                tc.tile_pool(name="ps_o", bufs=1, space="PSUM") as ps_o,
            ):
                sc_par = [0]  # running parity for score row-tiling
                for pi in range(NQB // 2):
                    qb0, qb1 = 2 * pi, 2 * pi + 1
                    rows = (plan[qb0], plan[qb1])
                    if not rows[0] and not rows[1]:
                        continue
                    maps = (
                        {tb: (mix, ql, qh) for tb, mix, ql, qh in rows[0]},
                        {tb: (mix, ql, qh) for tb, mix, ql, qh in rows[1]},
                    )
                    union = sorted(set(maps[0]) | set(maps[1]))
                    first = [min(m) if m else -1 for m in maps]
                    last = [max(m) if m else -1 for m in maps]
                    qsl = (
                        slice(qb0 * QB, (qb0 + 1) * QB),
                        slice(qb1 * QB, (qb1 + 1) * QB),
                    )
                    ya = [None, None]
                    yb = [None, None]
                    for j in (0, 1):
                        if rows[j]:
                            ya[j] = ypool.tile([128, QB], BF, tag=f"ya{j}", name=f"ya{j}")
                            yb[j] = ypool.tile([DH + 1, QB], BF, tag=f"yb{j}", name=f"yb{j}")
                            nc.vector.memset(yb[j][DH : DH + 1, :], 1.0)

                    for h in range(HPC):
                        ytp = [None, None]
                        for j in (0, 1):
                            if rows[j]:
                                ytp[j] = ps_y.tile(
                                    [DH + 1, QB], F32,
                                    tag=f"ytp{j}", name=f"ytp{j}",
                                )
                        pending = None  # (ea, [(j, tb, esl)...]) awaiting attn@V

                        def _flush():
                            nonlocal pending
                            if pending is None:
                                return
                            ea_p, parts = pending
                            for j, tb_p, esl in parts:
                                ql = esl[0] - j * QB
                                qh = esl[1] - j * QB
                                nc.tensor.matmul(
                                    ytp[j][:, ql:qh],
                                    v_all[:, tb_p, h, :],
                                    ea_p[:, esl[0] : esl[1]],
                                    start=(tb_p == first[j]),
                                    stop=(tb_p == last[j]),
                                )
                            pending = None

                        for ti_idx, tb in enumerate(union):
                            js = [j for j in (0, 1) if tb in maps[j]]
                            sps = ps_s.tile([128, 2 * QB], F32, tag="sps")
                            for j in js:
                                _mix, ql, qh = maps[j][tb]
                                off = 64 * (sc_par[0] & 1)
                                sc_par[0] += 1
                                nc.tensor.matmul(
                                    sps[:, j * QB + ql : j * QB + qh],
                                    kt_sb[
                                        off : off + 64,
                                        h,
                                        tb * TB : (tb + 1) * TB,
                                    ],
                                    qt_sb[
                                        off : off + 64,
                                        h,
                                        qsl[j].start + ql : qsl[j].start + qh,
                                    ],
                                    start=True,
                                    stop=True,
                                )
                            ea = epool.tile([128, 2 * QB], BF, tag="ea")
                            lo = min(js) * QB + maps[min(js)][tb][1]
                            hi = max(js) * QB + maps[max(js)][tb][2]
                            nc.scalar.activation(
                                ea[:, lo:hi],
                                sps[:, lo:hi],
                                mybir.ActivationFunctionType.Exp,
                                scale=float(1.0 / np.sqrt(DH)),
                            )
                            parts = []
                            for j in js:
                                mix, ql, qh = maps[j][tb]
                                esl = (j * QB + ql, j * QB + qh)
                                if mix is not None:
                                    nc.vector.tensor_mul(
                                        ea[:, esl[0] : esl[1]],
                                        ea[:, esl[0] : esl[1]],
                                        mk_sb[:, mix, ql:qh],
                                    )
                                parts.append((j, tb, esl))
                            _flush()
                            pending = (ea, parts)
                        _flush()

                        # normalize: copy PSUM out early (frees the bank),
                        # then rec = approx 1/denom, partition-broadcast it
                        # via DMA, and scale.
                        for j in (0, 1):
                            if not rows[j]:
                                continue
                            yraw = ypool.tile(
                                [DH + 1, QB], F32, tag=f"yraw{j}"
                            )
                            nc.vector.tensor_copy(yraw, ytp[j])
                            # 1/denom on ACT as exp(-ln(x)): both functions
                            # live in the natural_log_exp_and_others table
                            # set, so no table switching vs the score exps.
                            lg = rpool.tile([1, QB], F32, tag=f"lg{j}")
                            nc.scalar.activation(
                                lg, yraw[DH : DH + 1, :],
                                mybir.ActivationFunctionType.Ln,
                            )
                            rec = rpool.tile([1, QB], F32, tag=f"rec{j}")
                            nc.scalar.activation(
                                rec, lg,
                                mybir.ActivationFunctionType.Exp,
                                scale=-1.0,
                            )
                            # broadcast 1/denom across partitions via a K=1
                            # matmul into the bank ytp[j] just vacated
                            rps = ps_y.tile(
                                [DH + 1, QB], F32,
                                tag=f"ytp{j}", name=f"rps{j}",
                            )
                            nc.tensor.matmul(
                                rps[0:DH, :], ones_sb, rec,
                                start=True, stop=True,
                            )
                            dsty = (
                                ya[j][h * DH : (h + 1) * DH, :]
                                if h < 2
                                else yb[j][0:DH, :]
                            )
                            nc.vector.tensor_mul(
                                dsty, yraw[0:DH, :], rps[0:DH, :]
                            )

                    # output projection for both query blocks of the pair
                    for j in (0, 1):
                        if not rows[j]:
                            continue
                        qb = (qb0, qb1)[j]
                        for ss in range(QB // 128):
                            ssl = slice(ss * 128, (ss + 1) * 128)
                            po = ps_o.tile([128, E], F32, tag="po")
                            for nsl in (slice(0, 512), slice(512, E)):
                                nc.tensor.matmul(
                                    po[:, nsl], ya[j][:, ssl], wo0[:, nsl],
                                    start=True, stop=False,
                                )
                                nc.tensor.matmul(
                                    po[:, nsl], yb[j][:, ssl], wo1[:, nsl],
                                    start=False, stop=True,
                                )
                            osb = opool.tile([128, E], F32, tag="osb")
                            nc.vector.tensor_copy(osb, po)
                            nc.sync.dma_start(
                                out=out[
                                    qb * QB + ss * 128 : qb * QB + (ss + 1) * 128,
                                    :,
                                ],
                                in_=osb,
                            )
    return _split_waits(nc)


_CACHE = {}


def _get_kernel(plan_key, plan, n_mixed):
    if plan_key not in _CACHE:
        _CACHE[plan_key] = _build(plan, n_mixed)
    return _CACHE[plan_key]


# ---------------------------------------------------------------------------
def kernel(query, key, value, attn_mask, Wq, bq, Wk, bk, Wv, bv, Wo, bo):
    query = np.asarray(query)
    key = np.asarray(key)
    value = np.asarray(value)
    attn_mask = np.asarray(attn_mask)
    Wq, bq = np.asarray(Wq), np.asarray(bq)
    Wk, bk = np.asarray(Wk), np.asarray(bk)
    Wv, bv = np.asarray(Wv), np.asarray(bv)
    Wo, bo = np.asarray(Wo), np.asarray(bo)

    plan, packed = _mask_plan(attn_mask)
    n_mixed = sum(1 for row in plan for (_tb, mix, _ql, _qh) in row if mix is not None)
    plan_key = tuple(tuple(row) for row in plan)
    nc = _get_kernel(plan_key, plan, n_mixed)

    # per-batch transposed bf16 activations
    qT = [np.ascontiguousarray(query[n].T).astype(BF16) for n in range(N)]
    kT = [np.ascontiguousarray(key[n].T).astype(BF16) for n in range(N)]
    vT = [np.ascontiguousarray(value[n].T).astype(BF16) for n in range(N)]

    in_maps = []
    for core in range(8):
        n = core // 4
        g = core % 4
        cols = slice(g * NH3, (g + 1) * NH3)
        wo_aug = np.concatenate(
            [Wo[:, cols].T, (bo / 4.0)[None, :]], axis=0
        ).astype(BF16)
        in_maps.append(
            {
                "qt": qT[n],
                "kt": kT[n],
                "vt": vT[n],
                "wq": np.ascontiguousarray(Wq[cols, :].T).astype(BF16),
                "wk": np.ascontiguousarray(Wk[cols, :].T).astype(BF16),
                "wv": np.ascontiguousarray(Wv[cols, :].T).astype(BF16),
                "bqkv": np.stack(
                    [bq[cols], bk[cols], bv[cols]]
                ).astype(np.float32),
                "wo": wo_aug,
                "maskt": packed,
            }
        )

    trace = bool(int(os.environ.get("KERNEL_TRACE", "0")))
    res = run_bass_kernel_spmd(nc, in_maps, list(range(8)), trace=trace)
    kernel.last_exec_time_ns = res.exec_time_ns

    full = np.empty((N, S, E), np.float32)
    for n in range(N):
        acc = res.results[n * 4]["out"].astype(np.float32)
        for g in range(1, 4):
            acc = acc + res.results[n * 4 + g]["out"]
        full[n] = acc
    return full

